# revision 26
# baseline (speedup 1.0000x reference)
"""Trainium2 Bass kernel for nn_MeshEdgeBlock (GNN edge-block message passing).

Computes, per edge e with endpoints (s, d):
    x  = concat([src_nodes[s], dst_nodes[d], edge_feat[e]])   # [384]
    h  = silu(x @ W1 + b1)                                    # [512]
    y  = h @ W2 + b2                                          # [128]
    y  = LayerNorm(y) * gamma + beta + edge_feat[e]           # [128]

Sharding: edges assigned to the 8 cores by SRC-NODE RANGE (12500 rows per
core); within a core, edges are bucketed by dst-table chunk (4 chunks of
25000 rows, the int16 dma_gather index range) and sorted by src id.

Gather strategy: the two per-edge random gathers are split asymmetrically.
 - dst side: GpSimd dma_gather in transpose mode, ONE op per 2048 edges
   (single queue, single_packet=False - the only configuration measured
   correct on HW; all groups padded to exactly 16 tiles so every gather is
   exactly 2048 rows).
 - src side: NO random DMA at all. Because edges are sorted by src id, each
   128-edge tile's src rows span < NB*128 consecutive table rows. The host
   uploads that window's CONTENT per tile (contiguous DMA at full rate) plus
   the in-window offset loc[e]; on-chip, a one-hot matrix built from
   iota/is_equal on VectorE selects the rows via NB accumulating matmuls on
   TensorE (gather-as-matmul), producing src features already transposed
   [feat, edge] for mm1.

Device-side dataflow per group (16 tiles of 128 edges):
  - 1 dma_gather (dst) -> dT [128 feat, 2048 edges] bf16
  - edge features: plain DMA (residual) + xbar DMA transpose -> eT (mm1)
  - per tile: window DMA -> wsb [128 row, NB, 128 feat]; loc broadcast
    (GpSimd partition_broadcast); NB is_equal one-hots (VectorE); NB
    matmuls -> sxT psum -> sx bf16 (ScalarE copy); mm1 12 matmuls; silu;
    mm2 4 matmuls; LN stats via bn_stats/bn_aggr
  - rsqrt(var+eps) once per group (exponent-bit seed + 2 Newton steps);
    normalize + residual fused in one affine_then_add; bf16 out written
    p-major, un-permuted on host
"""

import numpy as np
import ml_dtypes
from contextlib import ExitStack

import concourse.bass as bass
import concourse.tile as tile
from concourse import bacc, library_config, mybir
from concourse.bass_utils import run_bass_kernel_spmd

# Problem constants (hardcoded per spec)
N_CORES = 8
E_FULL = 250000
N_NODES = 100000
D = 128          # node/edge feature dim == LN dim
H = 512          # hidden dim
LN_EPS = 1e-5

SRC_R = N_NODES // N_CORES   # 12500 src rows per core
DCH = 25000                  # dst-table chunk rows (int16 gather range)
NDC = 4                      # dst chunks
GT = 16                      # tiles per group (gather = GT*128 = 2048 rows)

BF16 = mybir.dt.bfloat16
F32 = mybir.dt.float32
I32 = mybir.dt.int32
I16 = mybir.dt.int16

RSQRT_MAGIC = 0x5F3759DF

_PROGRAM_CACHE = {}
_LAYOUT = {}


def _rsqrt_batched(nc, stats, mg2, gt):
    """inv = rsqrt(var+eps), nmi = -mu*inv, batched over the group."""
    mu = mg2[:, 0:2 * gt:2]
    var = mg2[:, 1:2 * gt:2]
    veps = stats.tile([128, GT], F32, tag="veps")
    nc.vector.tensor_scalar(out=veps[:, :gt], in0=var, scalar1=LN_EPS,
                            scalar2=None, op0=mybir.AluOpType.add)
    hv = stats.tile([128, GT], F32, tag="hv")
    nc.vector.tensor_scalar(out=hv[:, :gt], in0=veps[:, :gt], scalar1=-0.5,
                            scalar2=None, op0=mybir.AluOpType.mult)
    sh = stats.tile([128, GT], I32, tag="sh")
    nc.vector.tensor_scalar(out=sh[:, :gt], in0=veps[:, :gt].bitcast(I32),
                            scalar1=1, scalar2=None,
                            op0=mybir.AluOpType.arith_shift_right)
    seed = stats.tile([128, GT], I32, tag="seed")
    nc.vector.tensor_scalar(out=seed[:, :gt], in0=sh[:, :gt], scalar1=-1,
                            scalar2=RSQRT_MAGIC,
                            op0=mybir.AluOpType.mult,
                            op1=mybir.AluOpType.add)
    y = seed[:, :gt].bitcast(F32)
    for it in range(2):
        a = stats.tile([128, GT], F32, tag=f"nr_a{it}")
        nc.vector.tensor_mul(out=a[:, :gt], in0=y, in1=y)
        b = stats.tile([128, GT], F32, tag=f"nr_b{it}")
        nc.vector.tensor_mul(out=b[:, :gt], in0=a[:, :gt], in1=hv[:, :gt])
        ynew = stats.tile([128, GT], F32, tag=f"nr_y{it}")
        nc.vector.scalar_tensor_tensor(out=ynew[:, :gt], in0=b[:, :gt],
                                       scalar=1.5, in1=y,
                                       op0=mybir.AluOpType.add,
                                       op1=mybir.AluOpType.mult)
        y = ynew[:, :gt]
    nmi = stats.tile([128, GT], F32, tag="nmi")
    nc.vector.scalar_tensor_tensor(out=nmi[:, :gt], in0=mu, scalar=-1.0,
                                   in1=y, op0=mybir.AluOpType.mult,
                                   op1=mybir.AluOpType.mult)
    return y, nmi[:, :gt]


def _build_program(trivial_affine: bool, repeats: int = 1):
    """Build (and cache) the Bass program for the layout in _LAYOUT."""
    NB = _LAYOUT["NB"]
    t_b = tuple(_LAYOUT["t_b"])      # tiles per dst-chunk bucket (mult of GT)
    key = (trivial_affine, repeats, NB, t_b)
    if key in _PROGRAM_CACHE:
        return _PROGRAM_CACHE[key]

    NT = sum(t_b)
    NG = NT // GT
    EC2 = NT * 128

    nc = bacc.Bacc("TRN2", target_bir_lowering=False, debug=False,
                   num_devices=N_CORES)

    dnodes = nc.dram_tensor("dnodes", [N_NODES, D], BF16, kind="ExternalInput").ap()
    edges = nc.dram_tensor("edges", [EC2, D], BF16, kind="ExternalInput").ap()
    didx = nc.dram_tensor("didx", [NG * 128, GT * 8], I16, kind="ExternalInput").ap()
    win = nc.dram_tensor("win", [NG * 128, GT * NB * D], BF16, kind="ExternalInput").ap()
    locd = nc.dram_tensor("locd", [NG, GT * 128], F32, kind="ExternalInput").ap()
    iot = nc.dram_tensor("iot", [128, NB], F32, kind="ExternalInput").ap()
    w1 = nc.dram_tensor("w1", [D, 12 * D], BF16, kind="ExternalInput").ap()
    w2 = nc.dram_tensor("w2", [D, 4 * D], BF16, kind="ExternalInput").ap()
    out = nc.dram_tensor("out", [EC2, D], BF16, kind="ExternalOutput").ap()
    if not trivial_affine:
        b1d = nc.dram_tensor("b1d", [D, 4], F32, kind="ExternalInput").ap()
        b2d = nc.dram_tensor("b2d", [D, D], F32, kind="ExternalInput").ap()
        gmd = nc.dram_tensor("gmd", [D, D], F32, kind="ExternalInput").ap()
        btd = nc.dram_tensor("btd", [D, D], F32, kind="ExternalInput").ap()

    with tile.TileContext(nc) as tc, ExitStack() as ctx:
        const = ctx.enter_context(tc.tile_pool(name="const", bufs=1))
        io = ctx.enter_context(tc.tile_pool(name="io", bufs=4))
        idxp = ctx.enter_context(tc.tile_pool(name="idx", bufs=5))
        gtp = ctx.enter_context(tc.tile_pool(name="gt", bufs=5))
        wp = ctx.enter_context(tc.tile_pool(name="wp", bufs=3))
        ohp = ctx.enter_context(tc.tile_pool(name="oh", bufs=3))
        htp = ctx.enter_context(tc.tile_pool(name="ht", bufs=3))
        stats = ctx.enter_context(tc.tile_pool(name="stats", bufs=2))
        ps_sx = ctx.enter_context(tc.tile_pool(name="ps_sx", bufs=2, space="PSUM"))
        ps_ht = ctx.enter_context(tc.tile_pool(name="ps_ht", bufs=3, space="PSUM"))
        ps_y = ctx.enter_context(tc.tile_pool(name="ps_y", bufs=3, space="PSUM"))

        # dma_gather lives in the dynamically-loaded 'mlp' Q7 library
        nc.gpsimd.load_library(library_config.mlp)

        w1sb = const.tile([D, 12 * D], BF16)
        nc.sync.dma_start(out=w1sb[:], in_=w1[:])
        w2sb = const.tile([D, 4 * D], BF16)
        nc.sync.dma_start(out=w2sb[:], in_=w2[:])
        iots = const.tile([128, NB], F32)
        nc.sync.dma_start(out=iots[:], in_=iot[:])
        if not trivial_affine:
            b1sb = const.tile([D, 4], F32)
            nc.sync.dma_start(out=b1sb[:], in_=b1d[:])
            b2sb = const.tile([D, D], F32)
            nc.sync.dma_start(out=b2sb[:], in_=b2d[:])
            gmsb = const.tile([D, D], F32)
            nc.sync.dma_start(out=gmsb[:], in_=gmd[:])
            btsb = const.tile([D, D], F32)
            nc.sync.dma_start(out=btsb[:], in_=btd[:])

        def _group(g, dc, toff):
            base = toff * 128
            ne = GT * 128
            it_d = idxp.tile([128, GT * 8], I16, tag="didx")
            nc.sync.dma_start(out=it_d[:], in_=didx[g * 128:(g + 1) * 128, :])
            dT = gtp.tile([128, 1, GT * 128], BF16, tag="dT")
            nc.gpsimd.dma_gather(
                dT[:], dnodes[dc * DCH:(dc + 1) * DCH, :],
                it_d[:], ne, ne, D, transpose=True, single_packet=False)
            e_bf = io.tile([128, GT, D], BF16, tag="e")
            nc.scalar.dma_start(
                out=e_bf[:],
                in_=edges[base:base + ne, :].rearrange("(g p) f -> p g f", p=128))
            eT = gtp.tile([128, GT * 128], BF16, tag="eT")
            nc.sync.dma_start(out=eT[:], in_=edges[base:base + ne, :],
                              transpose=True)
            loc_sb = idxp.tile([1, GT * 128], F32, tag="loc")
            nc.sync.dma_start(out=loc_sb[:], in_=locd[g:g + 1, :])
            # whole group's src windows, host-laid p-major: one contiguous
            # read per partition
            wg = wp.tile([128, GT, NB, D], BF16, tag="wsb")
            nc.sync.dma_start(
                out=wg[:].rearrange("p t b f -> p (t b f)"),
                in_=win[g * 128:(g + 1) * 128, :])
            ysb = io.tile([128, GT, D], F32, tag="ysb")
            yout = io.tile([128, GT, D], BF16, tag="yout")
            mg2 = stats.tile([128, 2 * GT], F32, tag="mg2")

            for t in range(GT):
                T = toff + t
                co = t * 128
                # one-hot selection: oh_k[r, e] = (loc[e] == k*128 + r)
                locB = ohp.tile([128, D], F32, tag="locB")
                nc.gpsimd.partition_broadcast(
                    locB[:], loc_sb[0:1, t * 128:(t + 1) * 128])
                oh = ohp.tile([128, NB, D], BF16, tag="oh")
                for k in range(NB):
                    nc.vector.tensor_scalar(out=oh[:, k, :], in0=locB[:],
                                            scalar1=iots[:, k:k + 1],
                                            scalar2=None,
                                            op0=mybir.AluOpType.is_equal)
                sxps = ps_sx.tile([128, D], F32)
                for k in range(NB):
                    nc.tensor.matmul(out=sxps[:], lhsT=wg[:, t, k, :],
                                     rhs=oh[:, k, :],
                                     start=(k == 0), stop=(k == NB - 1))
                sx = wp.tile([128, D], BF16, tag="sx")
                nc.scalar.activation(out=sx[:], in_=sxps[:],
                                     func=mybir.ActivationFunctionType.Copy)

                rhs3 = (sx[:], dT[:, 0, co:co + 128], eT[:, co:co + 128])
                htps = ps_ht.tile([128, H], F32)
                for m in range(4):
                    for c in range(3):
                        nc.tensor.matmul(
                            out=htps[:, m * D:(m + 1) * D],
                            lhsT=w1sb[:, (c * 4 + m) * D:(c * 4 + m + 1) * D],
                            rhs=rhs3[c],
                            start=(c == 0), stop=(c == 2))

                ht = htp.tile([128, H], BF16)
                if trivial_affine:
                    nc.scalar.activation(out=ht[:], in_=htps[:],
                                         func=mybir.ActivationFunctionType.Silu)
                else:
                    for m in range(4):
                        nc.scalar.activation(
                            out=ht[:, m * D:(m + 1) * D],
                            in_=htps[:, m * D:(m + 1) * D],
                            func=mybir.ActivationFunctionType.Silu,
                            bias=b1sb[:, m:m + 1])

                yps = ps_y.tile([128, D], F32)
                for m in range(4):
                    nc.tensor.matmul(
                        out=yps[:],
                        lhsT=ht[:, m * D:(m + 1) * D],
                        rhs=w2sb[:, m * D:(m + 1) * D],
                        start=(m == 0), stop=(m == 3))

                if not trivial_affine:
                    nc.vector.tensor_add(out=ysb[:, t, :], in0=yps[:], in1=b2sb[:])
                else:
                    nc.scalar.activation(out=ysb[:, t, :], in_=yps[:],
                                         func=mybir.ActivationFunctionType.Copy)

                st6 = stats.tile([128, 6], F32, tag="st6")
                nc.vector.bn_stats(out=st6[:], in_=ysb[:, t, :])
                nc.vector.bn_aggr(out=mg2[:, 2 * t:2 * t + 2], in_=st6[:])

            inv, nmi = _rsqrt_batched(nc, stats, mg2[:, :2 * GT], GT)
            for t in range(GT):
                if trivial_affine:
                    nc.vector.affine_then_add(
                        out=yout[:, t, :], in0=ysb[:, t, :], in1=e_bf[:, t, :],
                        scale=inv[:, t:t + 1], bias=nmi[:, t:t + 1])
                else:
                    yn = io.tile([128, D], F32, tag="yn")
                    nc.vector.tensor_scalar(out=yn[:], in0=ysb[:, t, :],
                                            scalar1=inv[:, t:t + 1],
                                            scalar2=nmi[:, t:t + 1],
                                            op0=mybir.AluOpType.mult,
                                            op1=mybir.AluOpType.add)
                    nc.vector.tensor_mul(out=yn[:], in0=yn[:], in1=gmsb[:])
                    nc.vector.tensor_add(out=yn[:], in0=yn[:], in1=btsb[:])
                    nc.vector.tensor_add(out=yout[:, t, :], in0=yn[:], in1=e_bf[:, t, :])

            # p-major row order (row = p*GT + t): contiguous per-partition
            # writes; the host un-permutes via perms
            nc.scalar.dma_start(
                out=out[base:base + ne, :].rearrange("(p g) f -> p g f", g=GT),
                in_=yout[:])

        def _body():
            toff = 0
            g = 0
            for dc in range(NDC):
                for _ in range(t_b[dc] // GT):
                    _group(g, dc, toff)
                    toff += GT
                    g += 1

        if repeats == 1:
            _body()
        else:
            with tc.For_i(0, repeats, 1):
                _body()

    nc.compile()
    _PROGRAM_CACHE[key] = nc
    return nc


def _prep(inputs):
    """Host-side src-range sharding, dst bucketing, window construction."""
    f = {k: np.asarray(v) for k, v in inputs.items()}
    bf = ml_dtypes.bfloat16

    src_bf = f["src_node_features"].astype(bf)
    dst_bf = f["dst_node_features"].astype(bf)
    e_bf = f["edge_features"].astype(bf)
    si = f["src_indices"].astype(np.int64)
    di = f["dst_indices"].astype(np.int64)
    E = e_bf.shape[0]

    core_of = si // SRC_R
    dc_of = di // DCH

    # per (core, dc): edge id lists sorted by src id
    lists = []
    maxspan = 0
    for core in range(N_CORES):
        row = []
        sel = np.nonzero(core_of == core)[0]
        for dc in range(NDC):
            m = sel[dc_of[sel] == dc]
            m = m[np.argsort(si[m], kind="stable")]
            row.append(m)
            # track max 128-edge tile span for NB sizing
            sl = si[m] - core * SRC_R
            for p0 in range(0, len(sl), 128):
                seg = sl[p0:p0 + 128]
                if len(seg) > 1:
                    maxspan = max(maxspan, int(seg[-1] - seg[0]))
        lists.append(row)

    NB = max(3, -(-(maxspan + 1) // 128))
    t_b = []
    for dc in range(NDC):
        mx = max(len(lists[c][dc]) for c in range(N_CORES))
        tb = -(-mx // 128)
        tb = -(-tb // GT) * GT            # round up to whole groups
        t_b.append(tb)
    NT = sum(t_b)
    NG = NT // GT
    EC2 = NT * 128
    _LAYOUT["NB"] = NB
    _LAYOUT["t_b"] = t_b

    W1 = f["W1"].astype(np.float32)
    W2 = f["W2"].astype(np.float32)
    w1b = np.concatenate(
        [W1[c * D:(c + 1) * D, m * D:(m + 1) * D] for c in range(3) for m in range(4)],
        axis=1).astype(bf)
    w2b = np.concatenate([W2[m * D:(m + 1) * D, :] for m in range(4)], axis=1).astype(bf)

    b1 = f["b1"].astype(np.float32)
    b2 = f["b2"].astype(np.float32)
    gm = f["ln_gamma"].astype(np.float32)
    bt = f["ln_beta"].astype(np.float32)
    trivial = (not b1.any()) and (not b2.any()) and (not bt.any()) and bool(np.all(gm == 1.0))

    iota = np.zeros((128, NB), np.float32)
    for k in range(NB):
        iota[:, k] = np.arange(128) + k * 128

    in_maps, perms = [], []
    for core in range(N_CORES):
        slab = np.vstack([src_bf[core * SRC_R:(core + 1) * SRC_R],
                          np.zeros((NB * 128, D), bf)])
        e_core = np.zeros((EC2, D), bf)
        p_core = np.full((EC2,), -1, np.int64)
        dl_core = np.zeros((EC2,), np.int16)
        winb = np.zeros((NT, NB, 128, D), bf)
        locb = np.zeros((NT, 128), np.float32)
        toff = 0
        for dc in range(NDC):
            lst = lists[core][dc]
            pos = 0
            for tl in range(t_b[dc]):
                T = toff + tl
                base = T * 128
                seg = lst[pos:pos + 128]
                pos += 128
                k = len(seg)
                wb = 0
                if k:
                    sl = si[seg] - core * SRC_R
                    wb = int(sl[0])
                    locb[T, :k] = sl - wb
                    e_core[base:base + k] = e_bf[seg]
                    dl_core[base:base + k] = (di[seg] - dc * DCH).astype(np.int16)
                    # out rows are p-major within the group: row =
                    # group_base + lane*GT + tile_in_group
                    gb = (T // GT) * GT * 128
                    ti = T % GT
                    p_core[gb + np.arange(k) * GT + ti] = seg
                winb[T] = slab[wb:wb + NB * 128].reshape(NB, 128, D)
            toff += t_b[dc]

        # wrap dst indices into dma_gather's 16-partition layout, replicated
        # 8x down the 128 partitions (one copy per Q7 core)
        didx_b = np.zeros((NG, 128, GT * 8), np.int16)
        for g in range(NG):
            blk = dl_core[g * GT * 128:(g + 1) * GT * 128]
            didx_b[g] = np.tile(blk.reshape(GT * 8, 16).T, (8, 1))

        m = {
            "dnodes": dst_bf,
            "edges": e_core,
            "didx": didx_b.reshape(NG * 128, GT * 8),
            "win": np.ascontiguousarray(
                winb.reshape(NG, GT, NB, 128, D).transpose(0, 3, 1, 2, 4)
            ).reshape(NG * 128, GT * NB * D),
            "locd": locb.reshape(NG, GT * 128),
            "iot": iota,
            "w1": w1b,
            "w2": w2b,
        }
        if not trivial:
            m["b1d"] = np.ascontiguousarray(b1.reshape(4, D).T.astype(np.float32))
            m["b2d"] = np.broadcast_to(b2, (D, D)).copy()
            m["gmd"] = np.broadcast_to(gm, (D, D)).copy()
            m["btd"] = np.broadcast_to(bt, (D, D)).copy()
        in_maps.append(m)
        perms.append(p_core)
    return in_maps, trivial, perms


def kernel(**inputs) -> np.ndarray:
    in_maps, trivial, perms = _prep(inputs)
    nc = _build_program(trivial)
    res = run_bass_kernel_spmd(nc, in_maps, core_ids=list(range(N_CORES)))
    E = np.asarray(inputs["edge_features"]).shape[0]
    out = np.empty((E, D), np.float32)
    for core in range(N_CORES):
        o = np.asarray(res.results[core]["out"]).astype(np.float32)
        p = perms[core]
        valid = p >= 0
        out[p[valid]] = o[valid]
    return out


# revision 31
# speedup vs baseline: 1.1674x; 1.1674x over previous
"""Trainium2 Bass kernel for nn_MeshEdgeBlock (GNN edge-block message passing).

Computes, per edge e with endpoints (s, d):
    x  = concat([src_nodes[s], dst_nodes[d], edge_feat[e]])   # [384]
    h  = silu(x @ W1 + b1)                                    # [512]
    y  = h @ W2 + b2                                          # [128]
    y  = LayerNorm(y) * gamma + beta + edge_feat[e]           # [128]

Sharding: edges assigned to the 8 cores by SRC-NODE RANGE (12500 rows per
core); within a core, edges are bucketed by dst-table chunk (4 chunks of
25000 rows, the int16 dma_gather index range) and sorted by src id.

Gather strategy: the two per-edge random gathers are split asymmetrically.
 - dst side: GpSimd dma_gather in transpose mode, ONE op per 2048 edges
   (single queue, single_packet=False - the only configuration measured
   correct on HW; all groups padded to exactly 16 tiles so every gather is
   exactly 2048 rows).
 - src side: NO random DMA at all. Because edges are sorted by src id, each
   128-edge tile's src rows span < NB*128 consecutive table rows. The host
   uploads that window's CONTENT per tile (contiguous DMA at full rate) plus
   the in-window offset loc[e]; on-chip, a one-hot matrix built from
   iota/is_equal on VectorE selects the rows via NB accumulating matmuls on
   TensorE (gather-as-matmul), producing src features already transposed
   [feat, edge] for mm1.

Device-side dataflow per group (16 tiles of 128 edges):
  - 1 dma_gather (dst) -> dT [128 feat, 2048 edges] bf16
  - edge features: plain DMA (residual) + xbar DMA transpose -> eT (mm1)
  - per tile: window DMA -> wsb [128 row, NB, 128 feat]; loc broadcast
    (GpSimd partition_broadcast); NB is_equal one-hots (VectorE); NB
    matmuls -> sxT psum -> sx bf16 (ScalarE copy); mm1 12 matmuls; silu;
    mm2 4 matmuls; LN stats via bn_stats/bn_aggr
  - rsqrt(var+eps) once per group (exponent-bit seed + 2 Newton steps);
    normalize + residual fused in one affine_then_add; bf16 out written
    p-major, un-permuted on host
"""

import numpy as np
import ml_dtypes
from contextlib import ExitStack

import concourse.bass as bass
import concourse.tile as tile
from concourse import bacc, library_config, mybir
from concourse.bass_utils import run_bass_kernel_spmd

# Problem constants (hardcoded per spec)
N_CORES = 8
E_FULL = 250000
N_NODES = 100000
D = 128          # node/edge feature dim == LN dim
H = 512          # hidden dim
LN_EPS = 1e-5

SRC_R = N_NODES // N_CORES   # 12500 src rows per core
DCH = 25000                  # dst-table chunk rows (int16 gather range)
NDC = 4                      # dst chunks
GT = 16                      # tiles per group (gather = GT*128 = 2048 rows)

BF16 = mybir.dt.bfloat16
F32 = mybir.dt.float32
I32 = mybir.dt.int32
I16 = mybir.dt.int16

RSQRT_MAGIC = 0x5F3759DF

_PROGRAM_CACHE = {}
_LAYOUT = {}


def _rsqrt_batched(nc, stats, mg2, gt):
    """inv = rsqrt(var+eps), nmi = -mu*inv, batched over the group."""
    mu = mg2[:, 0:2 * gt:2]
    var = mg2[:, 1:2 * gt:2]
    veps = stats.tile([128, GT], F32, tag="veps")
    nc.vector.tensor_scalar(out=veps[:, :gt], in0=var, scalar1=LN_EPS,
                            scalar2=None, op0=mybir.AluOpType.add)
    hv = stats.tile([128, GT], F32, tag="hv")
    nc.vector.tensor_scalar(out=hv[:, :gt], in0=veps[:, :gt], scalar1=-0.5,
                            scalar2=None, op0=mybir.AluOpType.mult)
    sh = stats.tile([128, GT], I32, tag="sh")
    nc.vector.tensor_scalar(out=sh[:, :gt], in0=veps[:, :gt].bitcast(I32),
                            scalar1=1, scalar2=None,
                            op0=mybir.AluOpType.arith_shift_right)
    seed = stats.tile([128, GT], I32, tag="seed")
    nc.vector.tensor_scalar(out=seed[:, :gt], in0=sh[:, :gt], scalar1=-1,
                            scalar2=RSQRT_MAGIC,
                            op0=mybir.AluOpType.mult,
                            op1=mybir.AluOpType.add)
    y = seed[:, :gt].bitcast(F32)
    for it in range(2):
        a = stats.tile([128, GT], F32, tag=f"nr_a{it}")
        nc.vector.tensor_mul(out=a[:, :gt], in0=y, in1=y)
        b = stats.tile([128, GT], F32, tag=f"nr_b{it}")
        nc.vector.tensor_mul(out=b[:, :gt], in0=a[:, :gt], in1=hv[:, :gt])
        ynew = stats.tile([128, GT], F32, tag=f"nr_y{it}")
        nc.vector.scalar_tensor_tensor(out=ynew[:, :gt], in0=b[:, :gt],
                                       scalar=1.5, in1=y,
                                       op0=mybir.AluOpType.add,
                                       op1=mybir.AluOpType.mult)
        y = ynew[:, :gt]
    nmi = stats.tile([128, GT], F32, tag="nmi")
    nc.vector.scalar_tensor_tensor(out=nmi[:, :gt], in0=mu, scalar=-1.0,
                                   in1=y, op0=mybir.AluOpType.mult,
                                   op1=mybir.AluOpType.mult)
    return y, nmi[:, :gt]


def _build_program(trivial_affine: bool, repeats: int = 1):
    """Build (and cache) the Bass program for the layout in _LAYOUT."""
    NB = _LAYOUT["NB"]
    t_b = tuple(_LAYOUT["t_b"])      # tiles per dst-chunk bucket (mult of GT)
    key = (trivial_affine, repeats, NB, t_b)
    if key in _PROGRAM_CACHE:
        return _PROGRAM_CACHE[key]

    NT = sum(t_b)
    NG = NT // GT
    EC2 = NT * 128

    nc = bacc.Bacc("TRN2", target_bir_lowering=False, debug=False,
                   num_devices=N_CORES)

    dnodes = nc.dram_tensor("dnodes", [N_NODES, D], BF16, kind="ExternalInput").ap()
    edges = nc.dram_tensor("edges", [EC2, D], BF16, kind="ExternalInput").ap()
    didx = nc.dram_tensor("didx", [NG * 128, GT * 8], I16, kind="ExternalInput").ap()
    win = nc.dram_tensor("win", [NG * 128, GT * NB * D], BF16, kind="ExternalInput").ap()
    locd = nc.dram_tensor("locd", [NG, GT * 128], F32, kind="ExternalInput").ap()
    iot = nc.dram_tensor("iot", [128, NB], F32, kind="ExternalInput").ap()
    w1 = nc.dram_tensor("w1", [D, 12 * D], BF16, kind="ExternalInput").ap()
    w2 = nc.dram_tensor("w2", [D, 4 * D], BF16, kind="ExternalInput").ap()
    out = nc.dram_tensor("out", [EC2, D], BF16, kind="ExternalOutput").ap()
    if not trivial_affine:
        b1d = nc.dram_tensor("b1d", [D, 4], F32, kind="ExternalInput").ap()
        b2d = nc.dram_tensor("b2d", [D, D], F32, kind="ExternalInput").ap()
        gmd = nc.dram_tensor("gmd", [D, D], F32, kind="ExternalInput").ap()
        btd = nc.dram_tensor("btd", [D, D], F32, kind="ExternalInput").ap()

    with tile.TileContext(nc) as tc, ExitStack() as ctx:
        const = ctx.enter_context(tc.tile_pool(name="const", bufs=1))
        io = ctx.enter_context(tc.tile_pool(name="io", bufs=4))
        idxp = ctx.enter_context(tc.tile_pool(name="idx", bufs=5))
        gtp = ctx.enter_context(tc.tile_pool(name="gt", bufs=5))
        wp = ctx.enter_context(tc.tile_pool(name="wp", bufs=4))
        ohp = ctx.enter_context(tc.tile_pool(name="oh", bufs=4))
        htp = ctx.enter_context(tc.tile_pool(name="ht", bufs=4))
        stats = ctx.enter_context(tc.tile_pool(name="stats", bufs=2))
        ps_sx = ctx.enter_context(tc.tile_pool(name="ps_sx", bufs=2, space="PSUM"))
        ps_ht = ctx.enter_context(tc.tile_pool(name="ps_ht", bufs=3, space="PSUM"))
        ps_y = ctx.enter_context(tc.tile_pool(name="ps_y", bufs=3, space="PSUM"))

        # dma_gather lives in the dynamically-loaded 'mlp' Q7 library
        nc.gpsimd.load_library(library_config.mlp)

        w1sb = const.tile([D, 12 * D], BF16)
        nc.sync.dma_start(out=w1sb[:], in_=w1[:])
        w2sb = const.tile([D, 4 * D], BF16)
        nc.sync.dma_start(out=w2sb[:], in_=w2[:])
        iots = const.tile([128, NB], F32)
        nc.sync.dma_start(out=iots[:], in_=iot[:])
        if not trivial_affine:
            b1sb = const.tile([D, 4], F32)
            nc.sync.dma_start(out=b1sb[:], in_=b1d[:])
            b2sb = const.tile([D, D], F32)
            nc.sync.dma_start(out=b2sb[:], in_=b2d[:])
            gmsb = const.tile([D, D], F32)
            nc.sync.dma_start(out=gmsb[:], in_=gmd[:])
            btsb = const.tile([D, D], F32)
            nc.sync.dma_start(out=btsb[:], in_=btd[:])

        def _group(g, dc, toff):
            base = toff * 128
            ne = GT * 128
            it_d = idxp.tile([128, GT * 8], I16, tag="didx")
            nc.sync.dma_start(out=it_d[:], in_=didx[g * 128:(g + 1) * 128, :])
            dT = gtp.tile([128, 1, GT * 128], BF16, tag="dT")
            nc.gpsimd.dma_gather(
                dT[:], dnodes[dc * DCH:(dc + 1) * DCH, :],
                it_d[:], ne, ne, D, transpose=True, single_packet=False)
            e_bf = io.tile([128, GT, D], BF16, tag="e")
            nc.sync.dma_start(
                out=e_bf[:],
                in_=edges[base:base + ne, :].rearrange("(g p) f -> p g f", p=128))
            eT = gtp.tile([128, GT * 128], BF16, tag="eT")
            nc.sync.dma_start(out=eT[:], in_=edges[base:base + ne, :],
                              transpose=True)
            loc_sb = idxp.tile([1, GT * 128], F32, tag="loc")
            nc.sync.dma_start(out=loc_sb[:], in_=locd[g:g + 1, :])
            # whole group's src windows, host-laid p-major: one contiguous
            # read per partition
            wg = wp.tile([128, GT, NB, D], BF16, tag="wsb")
            nc.sync.dma_start(
                out=wg[:].rearrange("p t b f -> p (t b f)"),
                in_=win[g * 128:(g + 1) * 128, :])
            ysb = io.tile([128, GT, D], F32, tag="ysb")
            yout = io.tile([128, GT, D], BF16, tag="yout")
            mg2 = stats.tile([128, 2 * GT], F32, tag="mg2")

            for t in range(GT):
                T = toff + t
                co = t * 128
                # one-hot selection: oh_k[r, e] = (loc[e] == k*128 + r)
                locB = ohp.tile([128, D], F32, tag="locB")
                nc.gpsimd.partition_broadcast(
                    locB[:], loc_sb[0:1, t * 128:(t + 1) * 128])
                oh = ohp.tile([128, NB, D], BF16, tag="oh")
                for k in range(NB):
                    nc.vector.tensor_scalar(out=oh[:, k, :], in0=locB[:],
                                            scalar1=iots[:, k:k + 1],
                                            scalar2=None,
                                            op0=mybir.AluOpType.is_equal)
                sxps = ps_sx.tile([128, D], F32)
                for k in range(NB):
                    nc.tensor.matmul(out=sxps[:], lhsT=wg[:, t, k, :],
                                     rhs=oh[:, k, :],
                                     start=(k == 0), stop=(k == NB - 1))
                sx = wp.tile([128, D], BF16, tag="sx")
                nc.scalar.activation(out=sx[:], in_=sxps[:],
                                     func=mybir.ActivationFunctionType.Copy)

                rhs3 = (sx[:], dT[:, 0, co:co + 128], eT[:, co:co + 128])
                htps = ps_ht.tile([128, H], F32)
                for m in range(4):
                    for c in range(3):
                        nc.tensor.matmul(
                            out=htps[:, m * D:(m + 1) * D],
                            lhsT=w1sb[:, (c * 4 + m) * D:(c * 4 + m + 1) * D],
                            rhs=rhs3[c],
                            start=(c == 0), stop=(c == 2))

                ht = htp.tile([128, H], BF16)
                if trivial_affine:
                    nc.scalar.activation(out=ht[:], in_=htps[:],
                                         func=mybir.ActivationFunctionType.Silu)
                else:
                    for m in range(4):
                        nc.scalar.activation(
                            out=ht[:, m * D:(m + 1) * D],
                            in_=htps[:, m * D:(m + 1) * D],
                            func=mybir.ActivationFunctionType.Silu,
                            bias=b1sb[:, m:m + 1])

                yps = ps_y.tile([128, D], F32)
                for m in range(4):
                    nc.tensor.matmul(
                        out=yps[:],
                        lhsT=ht[:, m * D:(m + 1) * D],
                        rhs=w2sb[:, m * D:(m + 1) * D],
                        start=(m == 0), stop=(m == 3))

                if not trivial_affine:
                    nc.vector.tensor_add(out=ysb[:, t, :], in0=yps[:], in1=b2sb[:])
                else:
                    nc.scalar.activation(out=ysb[:, t, :], in_=yps[:],
                                         func=mybir.ActivationFunctionType.Copy)

                st6 = stats.tile([128, 6], F32, tag="st6")
                nc.vector.bn_stats(out=st6[:], in_=ysb[:, t, :])
                nc.vector.bn_aggr(out=mg2[:, 2 * t:2 * t + 2], in_=st6[:])

            inv, nmi = _rsqrt_batched(nc, stats, mg2[:, :2 * GT], GT)
            for t in range(GT):
                if trivial_affine:
                    nc.vector.affine_then_add(
                        out=yout[:, t, :], in0=ysb[:, t, :], in1=e_bf[:, t, :],
                        scale=inv[:, t:t + 1], bias=nmi[:, t:t + 1])
                else:
                    yn = io.tile([128, D], F32, tag="yn")
                    nc.vector.tensor_scalar(out=yn[:], in0=ysb[:, t, :],
                                            scalar1=inv[:, t:t + 1],
                                            scalar2=nmi[:, t:t + 1],
                                            op0=mybir.AluOpType.mult,
                                            op1=mybir.AluOpType.add)
                    nc.vector.tensor_mul(out=yn[:], in0=yn[:], in1=gmsb[:])
                    nc.vector.tensor_add(out=yn[:], in0=yn[:], in1=btsb[:])
                    nc.vector.tensor_add(out=yout[:, t, :], in0=yn[:], in1=e_bf[:, t, :])

            # p-major row order (row = p*GT + t): contiguous per-partition
            # writes; the host un-permutes via perms
            nc.sync.dma_start(
                out=out[base:base + ne, :].rearrange("(p g) f -> p g f", g=GT),
                in_=yout[:])

        def _body():
            toff = 0
            g = 0
            for dc in range(NDC):
                for _ in range(t_b[dc] // GT):
                    _group(g, dc, toff)
                    toff += GT
                    g += 1

        if repeats == 1:
            _body()
        else:
            with tc.For_i(0, repeats, 1):
                _body()

    nc.compile()
    _PROGRAM_CACHE[key] = nc
    return nc


def _prep(inputs):
    """Host-side src-range sharding, dst bucketing, window construction."""
    f = {k: np.asarray(v) for k, v in inputs.items()}
    bf = ml_dtypes.bfloat16

    src_bf = f["src_node_features"].astype(bf)
    dst_bf = f["dst_node_features"].astype(bf)
    e_bf = f["edge_features"].astype(bf)
    si = f["src_indices"].astype(np.int64)
    di = f["dst_indices"].astype(np.int64)
    E = e_bf.shape[0]

    core_of = si // SRC_R
    dc_of = di // DCH

    # per (core, dc): span-capped segments of <=128 src-sorted edges each.
    # Capping a tile's src span at SPAN rows keeps the one-hot window at
    # NB=2 blocks; only ~0.2% of tiles hit the span cut before 128 edges.
    SPAN = 256
    segs = []
    for core in range(N_CORES):
        row = []
        sel = np.nonzero(core_of == core)[0]
        for dc in range(NDC):
            m = sel[dc_of[sel] == dc]
            m = m[np.argsort(si[m], kind="stable")]
            sl = si[m] - core * SRC_R
            cuts = []
            pos = 0
            while pos < len(m):
                hi = min(pos + 128, len(m))
                lim = int(np.searchsorted(sl, sl[pos] + SPAN, side="left"))
                hi = min(hi, max(lim, pos + 1))
                cuts.append(m[pos:hi])
                pos = hi
            row.append(cuts)
        segs.append(row)

    NB = 2
    t_b = []
    for dc in range(NDC):
        mx = max(len(segs[c][dc]) for c in range(N_CORES))
        tb = -(-mx // GT) * GT            # round up to whole groups
        t_b.append(tb)
    NT = sum(t_b)
    NG = NT // GT
    EC2 = NT * 128
    _LAYOUT["NB"] = NB
    _LAYOUT["t_b"] = t_b

    W1 = f["W1"].astype(np.float32)
    W2 = f["W2"].astype(np.float32)
    w1b = np.concatenate(
        [W1[c * D:(c + 1) * D, m * D:(m + 1) * D] for c in range(3) for m in range(4)],
        axis=1).astype(bf)
    w2b = np.concatenate([W2[m * D:(m + 1) * D, :] for m in range(4)], axis=1).astype(bf)

    b1 = f["b1"].astype(np.float32)
    b2 = f["b2"].astype(np.float32)
    gm = f["ln_gamma"].astype(np.float32)
    bt = f["ln_beta"].astype(np.float32)
    trivial = (not b1.any()) and (not b2.any()) and (not bt.any()) and bool(np.all(gm == 1.0))

    iota = np.zeros((128, NB), np.float32)
    for k in range(NB):
        iota[:, k] = np.arange(128) + k * 128

    in_maps, perms = [], []
    for core in range(N_CORES):
        slab = np.vstack([src_bf[core * SRC_R:(core + 1) * SRC_R],
                          np.zeros((NB * 128, D), bf)])
        e_core = np.zeros((EC2, D), bf)
        p_core = np.full((EC2,), -1, np.int64)
        dl_core = np.zeros((EC2,), np.int16)
        winb = np.zeros((NT, NB, 128, D), bf)
        locb = np.zeros((NT, 128), np.float32)
        toff = 0
        for dc in range(NDC):
            cuts = segs[core][dc]
            for tl in range(t_b[dc]):
                T = toff + tl
                base = T * 128
                seg = cuts[tl] if tl < len(cuts) else np.zeros((0,), np.int64)
                k = len(seg)
                wb = 0
                if k:
                    sl = si[seg] - core * SRC_R
                    wb = int(sl[0])
                    locb[T, :k] = sl - wb
                    e_core[base:base + k] = e_bf[seg]
                    dl_core[base:base + k] = (di[seg] - dc * DCH).astype(np.int16)
                    # out rows are p-major within the group: row =
                    # group_base + lane*GT + tile_in_group
                    gb = (T // GT) * GT * 128
                    ti = T % GT
                    p_core[gb + np.arange(k) * GT + ti] = seg
                winb[T] = slab[wb:wb + NB * 128].reshape(NB, 128, D)
            toff += t_b[dc]

        # wrap dst indices into dma_gather's 16-partition layout, replicated
        # 8x down the 128 partitions (one copy per Q7 core)
        didx_b = np.zeros((NG, 128, GT * 8), np.int16)
        for g in range(NG):
            blk = dl_core[g * GT * 128:(g + 1) * GT * 128]
            didx_b[g] = np.tile(blk.reshape(GT * 8, 16).T, (8, 1))

        m = {
            "dnodes": dst_bf,
            "edges": e_core,
            "didx": didx_b.reshape(NG * 128, GT * 8),
            "win": np.ascontiguousarray(
                winb.reshape(NG, GT, NB, 128, D).transpose(0, 3, 1, 2, 4)
            ).reshape(NG * 128, GT * NB * D),
            "locd": locb.reshape(NG, GT * 128),
            "iot": iota,
            "w1": w1b,
            "w2": w2b,
        }
        if not trivial:
            m["b1d"] = np.ascontiguousarray(b1.reshape(4, D).T.astype(np.float32))
            m["b2d"] = np.broadcast_to(b2, (D, D)).copy()
            m["gmd"] = np.broadcast_to(gm, (D, D)).copy()
            m["btd"] = np.broadcast_to(bt, (D, D)).copy()
        in_maps.append(m)
        perms.append(p_core)
    return in_maps, trivial, perms


def kernel(**inputs) -> np.ndarray:
    in_maps, trivial, perms = _prep(inputs)
    nc = _build_program(trivial)
    res = run_bass_kernel_spmd(nc, in_maps, core_ids=list(range(N_CORES)))
    E = np.asarray(inputs["edge_features"]).shape[0]
    out = np.empty((E, D), np.float32)
    for core in range(N_CORES):
        o = np.asarray(res.results[core]["out"]).astype(np.float32)
        p = perms[core]
        valid = p >= 0
        out[p[valid]] = o[valid]
    return out
